# revision 15
# baseline (speedup 1.0000x reference)
"""Trainium2 Bass kernel for nn_CLFBlock (linear -> LIF scan -> linear -> T-mean -> log_softmax).

Self-contained: hardcodes shapes T=32, B=512, D=1024, C=1000 and data-parallel
sharding of the batch dim across 8 NeuronCores.

Math notes:
  h = x @ W1.T + b1                      (computed in fp8 on the PE, fp32 accum)
  LIF (tau=2, v_th=1, hard reset to 0):
     v' = 0.5*v + 0.5*h
     s  = (v' >= 1);  v = v' * (v' < 1)
  Scan state is the pre-reset voltage w_t, kept with h pre-halved:
  hh = 0.5*h + 0.5*b1, and per step (one fused DVE op on VectorE):
     w  = select(w < 1, w, 0) * 0.5 + hh
  The spike complement m_t = (w_t < 1) is produced on the otherwise-idle
  GPSIMD engine as fp8 so the tensor engine can accumulate two steps per
  DoubleRow matmul: msum_psum += [I;I] @ [m_t; m_t+1], and
  sum_t s_t = T - msum.
  y = mean_t(s_t @ W2.T + b2) = (sum_t s_t) @ W2.T / T + b2
  out = log_softmax(y, axis=1)

Layout: the tensor engine contracts along the partition axis; the host packs
every input into its exact SBUF layout so each DMA is a plain contiguous
[128, N] copy with multi-KB per-partition lines:
  xT  [128, 16384] : free = chunk(4) * dj(8) * 512   (tb-major inside chunk)
  W1T [128, 8192]  : free = j(8) * dj(8) * 128       (output-tile major)
  W2T [128, 8192]  : free = ej(8) * 1024 (padded from 1000)
mm1 runs in 9 t-groups (7x4 steps + 2x2 steps) so the LIF scan starts early
and ends nearly with mm1; msum pairs trail two groups behind in the tensor
stream so the PE never waits on the scan.  PSUM->SBUF h copies (cast, *0.5,
+0.5*b1) stay on the scalar (ACT) engine which reads PSUM cheaply.
"""

import numpy as np
from contextlib import ExitStack

import concourse.bass as bass
import concourse.tile as tile
from concourse import bacc, mybir
from concourse.bass_utils import run_bass_kernel_spmd

N_CORES = 8


def _lif_op():
    """Fused LIF step as a custom DVE op:
         out = select(in0 < s0, in0, 0) * s1 + in1
       i.e. w_new = reset(w_old)*0.5 + hh  in a single VectorE instruction."""
    from concourse import dve_ops
    from concourse.dve_spec import Spec, Src0, Src1, Zero, C0, C1, select, lower
    from concourse.dve_uop import DveOpSpec

    for op in dve_ops.OPS:
        if op.name == "LIF_STEP_ANT":
            return op
    spec = Spec(
        body=select(Src0 < C0, Src0, Zero) * C1 + Src1,
        reference=lambda in0, in1, s0, s1, imm2: (
            np.where(in0.astype(np.float32) < s0, in0.astype(np.float32), 0.0) * s1
            + in1.astype(np.float32)).astype(np.float32),
    )
    row = dve_ops._CUSTOM_DVE_ROW_BASE + len(dve_ops.OPS)
    shas = {}
    for ver in ("v3", "v4"):
        try:
            shas[ver] = DveOpSpec(name="LIF_STEP_ANT", opcode=row,
                                  uops=lower(spec, ver=ver), rd1_en=True).sha(ver)
        except Exception:
            pass
    op = dve_ops.DveOp("LIF_STEP_ANT", spec, subdim=False, uops_sha=shas)
    dve_ops.OPS.append(op)
    dve_ops._SUB_OPCODE_FOR_NAME[op.name] = row
    dve_ops.CUSTOM_DVE_SPECS[op.name] = spec
    return op


T, B, D, C = 32, 512, 1024, 1000
BC = B // N_CORES          # 64 rows per core
TB = T * BC                # 2048 matmul rows per core
FP32 = mybir.dt.float32
BF16 = mybir.dt.bfloat16
FP8 = mybir.dt.float8e4
W1_PRESCALE = 256.0   # host multiplies W1/W2 by this (exact power of 2) so the
                      # small uniform(-1/32,1/32) values stay in fp8e4m3's
                      # normal range; compensated in the h-copy / y-copy scale
AF = mybir.ActivationFunctionType
OP = mybir.AluOpType

# mm1 t-groups: (t0, tcount), aligned to the 512-col x chunks.  Small groups
# at the start (scan spin-up) and end (scan tail); full-chunk FD512 groups in
# the middle where the DoubleRow LDWEIGHTS is fully hidden by the matmul.
GROUPS = [(0, 4), (4, 4), (8, 8), (16, 8), (24, 4), (28, 2), (30, 2)]


def build_program():
    nc = bacc.Bacc("TRN2", target_bir_lowering=False, debug=False, num_devices=N_CORES)

    xt_d = nc.dram_tensor("xT", [128, 16 * 1024], FP8, kind="ExternalInput").ap()
    w1t_d = nc.dram_tensor("W1T", [128, 8 * 1024], FP8, kind="ExternalInput").ap()
    b1_d = nc.dram_tensor("b1", [D], FP32, kind="ExternalInput").ap()
    w2t_d = nc.dram_tensor("W2T", [128, 8 * 1024], FP8, kind="ExternalInput").ap()
    b2_d = nc.dram_tensor("b2", [C], FP32, kind="ExternalInput").ap()
    y_d = nc.dram_tensor("y", [BC, C], FP32, kind="ExternalOutput").ap()

    with tile.TileContext(nc) as tc, ExitStack() as ctx:
        persist = ctx.enter_context(tc.tile_pool(name="persist", bufs=1))
        small = ctx.enter_context(tc.tile_pool(name="small", bufs=1))
        ps_h = ctx.enter_context(tc.tile_pool(name="ps_h", bufs=5, space="PSUM"))
        ps_ms = ctx.enter_context(tc.tile_pool(name="ps_ms", bufs=1, space="PSUM"))
        ps_y = ctx.enter_context(tc.tile_pool(name="ps_y", bufs=2, space="PSUM"))

        # ---- input DMAs first so the HWDGE rings start streaming ASAP.
        # One ring sustains only ~150 GB/s, so the first-needed data (W1 j0/j1
        # + x chunk 0) is split across BOTH rings in need order.  The
        # scalar(ACT) ring gets few issues so the ACT queue is free for the
        # h-copies by the time mm1 group 0's PSUM is ready. ----
        xt = persist.tile([128, 16 * 1024], FP8)
        w1t = persist.tile([128, 8 * 1024], FP8)
        w2t = persist.tile([128, 8 * 1024], FP8)
        b1_sb = small.tile([128, 8], FP32)
        b2_sb = small.tile([1, C], FP32)

        nc.sync.dma_start(w1t[:, 0:1024], w1t_d[:, 0:1024])          # W1 j0
        nc.scalar.dma_start(b1_sb[:], b1_d.rearrange("(j p) -> p j", p=128))
        nc.scalar.dma_start(b2_sb[:], b2_d.rearrange("(a c) -> a c", a=1))
        nc.sync.dma_start(xt[:, 0:2048], xt_d[:, 0:2048])            # x c0 dj0-3
        nc.scalar.dma_start(xt[:, 2048:4096], xt_d[:, 2048:4096])    # x c0 dj4-7
        nc.sync.dma_start(w1t[:, 1024:2048], w1t_d[:, 1024:2048])    # W1 j1
        nc.scalar.dma_start(w1t[:, 4096:8192], w1t_d[:, 4096:8192])  # W1 j4-7
        nc.sync.dma_start(w1t[:, 2048:4096], w1t_d[:, 2048:4096])    # W1 j2-3
        nc.sync.dma_start(xt[:, 4096:8192], xt_d[:, 4096:8192])      # chunk 1
        nc.sync.dma_start(xt[:, 8192:12288], xt_d[:, 8192:12288])    # chunk 2
        nc.sync.dma_start(xt[:, 12288:16384], xt_d[:, 12288:16384])  # chunk 3
        nc.sync.dma_start(w2t[:], w2t_d[:])

        w1t4 = w1t[:].rearrange("p (j k e) -> p j k e", j=8, k=8)
        xt4 = xt[:].rearrange("p (c k t) -> p c k t", c=4, k=8)
        w2t3 = w2t[:].rearrange("p (e c) -> p e c", e=8)

        # ---- constants / biases (prologue, engines otherwise idle) ----
        io = small.tile([128, 128], mybir.dt.int32)
        nc.gpsimd.iota(io[:], pattern=[[1, 128]], base=0, channel_multiplier=-1)
        ones = small.tile([1, BC], BF16)
        nc.gpsimd.memset(ones[:], 1.0)
        ident = small.tile([128, 128], BF16)
        nc.vector.tensor_scalar(ident[:], io[:], 0, None, op0=OP.is_equal)

        b1h = small.tile([128, 8], FP32)
        nc.vector.tensor_scalar_mul(b1h[:], b1_sb[:], 0.5)
        b2_32 = small.tile([1, C], BF16)
        nc.scalar.activation(b2_32[:], b2_sb[:], AF.Copy,
                             scale=float(T) * W1_PRESCALE)
        warm = small.tile([1, 8], FP32)

        # ---- matmul1: h[e, tb] = W1 @ x.T, fused 0.5*h + 0.5*b1 into scan
        # layout via the ACT copy.  h_sb free index = t*512 + j*64 + b ----
        h_sb = persist.tile([128, T * 512], BF16)
        h3 = h_sb[:].rearrange("p (t x) -> p t x", x=512)

        def mm1_group(g, t0, tcnt):
            n = tcnt * 64
            c = (t0 * 64) // 512
            o = (t0 * 64) % 512
            for j in range(8):
                ps = ps_h.tile([128, 512], FP32, tag="ps_h", name=f"psh_{g}_{j}")
                for dp in range(4):   # pairs of contraction tiles (DoubleRow)
                    nc.tensor.matmul(
                        ps[:, 0:n],
                        w1t4[:, j, 2 * dp:2 * dp + 2, :],
                        xt4[:, c, 2 * dp:2 * dp + 2, o:o + n],
                        start=(dp == 0), stop=(dp == 3),
                        perf_mode=mybir.MatmulPerfMode.DoubleRow,
                    )
                nc.scalar.activation(
                    h3[:, t0:t0 + tcnt, j * 64:(j + 1) * 64],
                    ps[:, 0:n].rearrange("p (t b) -> p t b", t=tcnt),
                    AF.Identity, scale=0.5 / W1_PRESCALE, bias=b1h[:, j:j + 1],
                )

        # ---- spike-sum accumulation: msum += I @ m_t ----
        m_all = persist.tile([128, T * 512], BF16)
        msum = ps_ms.tile([128, 512], FP32)

        def ms_group(t0, tcnt):
            for t in range(t0, t0 + tcnt):
                nc.tensor.matmul(msum[:], ident[:],
                                 m_all[:, t * 512:(t + 1) * 512],
                                 start=(t == 0), stop=(t == T - 1))

        # ---- LIF scan: VectorE custom op + bf16 mask on the same queue ----
        lif = _lif_op()
        wst = small.tile([128, 512], BF16)
        nc.vector.memset(wst[:], 0.0)

        def scan_steps(t0, tcnt):
            for t in range(t0, t0 + tcnt):
                h_t = h_sb[:, t * 512:(t + 1) * 512]
                nc.vector._custom_dve(lif, out=wst[:], in0=wst[:], in1=h_t,
                                      s0=1.0, s1=0.5)
                nc.vector.tensor_scalar(m_all[:, t * 512:(t + 1) * 512],
                                        wst[:], 1.0, None, op0=OP.is_lt)

        # Emission is program order: scan steps trail mm1 by one group (their
        # h is complete), msum matmuls trail by two (their masks are complete),
        # so the tensor stream stays dense and never waits on the scan until
        # the very tail.
        ng = len(GROUPS)
        for g in range(ng):
            mm1_group(g, *GROUPS[g])
            if g == 3:
                # warm the Exp/Ln ACT tables mid-kernel (ACT has slack here)
                # so the epilogue doesn't pay the ~1.3us table-load switches.
                # Ln first so the pass leaves the exp set resident for Exp.
                nc.scalar.activation(warm[:, 4:8], b1_sb[0:1, 4:8], AF.Ln)
                nc.scalar.activation(warm[:, 0:4], b1_sb[0:1, 0:4], AF.Exp)
            if g >= 1:
                scan_steps(*GROUPS[g - 1])
            if g >= 2:
                ms_group(*GROUPS[g - 2])

        # mm2 bias rank-1 matmuls emitted here: no scan dependency, and they
        # keep the PE busy (p-state) while the scan tail drains.  The dummy
        # matmuls interleaved with the tail ms groups reread old mask slices
        # purely to keep the PE's p-state ramped for mm2.
        psy = [ps_y.tile([BC, 512], FP32, tag="ps_y", name=f"psy{h}")
               for h in range(2)]
        for half in range(2):
            n = 512 if half == 0 else C - 512
            c0 = half * 512
            nc.tensor.matmul(psy[half][:, 0:n], ones[:], b2_32[:, c0:c0 + n],
                             start=True, stop=False)

        def warm_mm():
            ps = ps_h.tile([128, 512], FP32, tag="ps_h", name="warm")
            nc.tensor.matmul(ps[:], ident[:], m_all[:, 0:512],
                             start=True, stop=True)

        scan_steps(*GROUPS[ng - 1])
        for t in range(GROUPS[ng - 2][0], 32):
            nc.tensor.matmul(msum[:], ident[:], m_all[:, t * 512:(t + 1) * 512],
                             start=False, stop=(t == T - 1))
            if t < T - 1:
                warm_mm()

        # sum_t s_t = T - msum; spike counts are small integers, ~exact in fp8
        ssum = small.tile([128, 512], FP8)
        nc.scalar.activation(ssum[:], msum[:], AF.Copy, scale=-1.0, bias=float(T))
        ssum3 = ssum[:].rearrange("p (j b) -> p j b", j=8)

        # ---- matmul2: y = ssum @ W2.T / T + b2 (DoubleRow fp8), kept in
        # PSUM; the epilogue reads PSUM directly (no y_sb staging copy) ----
        for half in range(2):
            n = 512 if half == 0 else C - 512
            c0 = half * 512
            for pj in range(4):
                nc.tensor.matmul(
                    psy[half][:, 0:n],
                    ssum3[:, 2 * pj:2 * pj + 2, :],
                    w2t3[:, 2 * pj:2 * pj + 2, c0:c0 + n],
                    start=False, stop=(pj == 3),
                    perf_mode=mybir.MatmulPerfMode.DoubleRow,
                )

        # ---- log_softmax over C (y is small enough that no max-shift is
        # needed: |y| <= D/T + |b2| ~ 32, exp stays in fp32 range).  The Exp
        # accumulator gives the row sums without a separate DVE reduce; the
        # final subtract fuses the 1/(T*PRESCALE) scale on the PSUM read. ----
        ysc = 1.0 / (T * W1_PRESCALE)
        ez = small.tile([BC, 1024], FP32)
        se = small.tile([BC, 2], FP32)
        nc.scalar.activation(ez[:, 0:512], psy[0][:, 0:512], AF.Exp,
                             scale=ysc, accum_out=se[:, 0:1])
        nc.scalar.activation(ez[:, 512:C], psy[1][:, 0:C - 512], AF.Exp,
                             scale=ysc, accum_out=se[:, 1:2])
        ssum_e = small.tile([BC, 1], FP32)
        nc.vector.tensor_scalar(ssum_e[:], se[:, 0:1], se[:, 1:2], None,
                                op0=OP.add)
        lse = small.tile([BC, 1], FP32)
        nc.scalar.activation(lse[:], ssum_e[:], AF.Ln)
        out_sb = small.tile([BC, C], FP32)
        nc.vector.tensor_scalar(out_sb[:, 0:512], psy[0][:, 0:512], ysc,
                                lse[:], op0=OP.mult, op1=OP.subtract)
        nc.sync.dma_start(y_d[:, 0:512], out_sb[:, 0:512])
        nc.vector.tensor_scalar(out_sb[:, 512:C], psy[1][:, 0:C - 512], ysc,
                                lse[:], op0=OP.mult, op1=OP.subtract)
        nc.sync.dma_start(y_d[:, 512:C], out_sb[:, 512:C])

    nc.compile()
    return nc


_CACHE = {}


def kernel(x, W1, b1, W2, b2):
    if "nc" not in _CACHE:
        _CACHE["nc"] = build_program()
    nc = _CACHE["nc"]

    f8 = mybir.dt.np(FP8)
    x = np.asarray(x, dtype=np.float32)
    # W1T packed [p, j(8), dj(8), 128]: W1T[dj*128+p, j*128+e'] -> [p][j][dj][e']
    w1f8 = (np.asarray(W1, dtype=np.float32).T * W1_PRESCALE).astype(f8)
    w1p = np.ascontiguousarray(
        w1f8.reshape(8, 128, 8, 128).transpose(1, 2, 0, 3).reshape(128, 8192))
    # W2T packed [p, ej(8), 1024 (padded from 1000)]
    w2f8 = (np.asarray(W2, dtype=np.float32).T * W1_PRESCALE).astype(f8)
    w2p = np.zeros((128, 8, 1024), dtype=f8)
    w2p[:, :, 0:C] = w2f8.reshape(8, 128, C).transpose(1, 0, 2)
    w2p = np.ascontiguousarray(w2p.reshape(128, 8192))
    b1 = np.ascontiguousarray(b1, dtype=np.float32)
    b2 = np.ascontiguousarray(b2, dtype=np.float32)

    in_maps = []
    for i in range(N_CORES):
        # xT packed [p, chunk(4), dj(8), 512]: xT[dj*128+p, c*512+t']
        xs8 = x[:, i * BC:(i + 1) * BC, :].reshape(TB, D).astype(f8)
        xs = np.ascontiguousarray(
            xs8.T.reshape(8, 128, 4, 512).transpose(1, 2, 0, 3).reshape(128, 16384))
        in_maps.append({"xT": xs, "W1T": w1p, "b1": b1, "W2T": w2p, "b2": b2})

    res = run_bass_kernel_spmd(nc, in_maps, core_ids=list(range(N_CORES)),
                               **_CACHE.get("run_kwargs", {}))
    _CACHE["last_results"] = res
    out = np.concatenate([res.results[i]["y"] for i in range(N_CORES)], axis=0)
    return out


# revision 20
# speedup vs baseline: 1.1311x; 1.1311x over previous
"""Trainium2 Bass kernel for nn_CLFBlock (linear -> LIF scan -> linear -> T-mean -> log_softmax).

Self-contained: hardcodes shapes T=32, B=512, D=1024, C=1000 and data-parallel
sharding of the batch dim across 8 NeuronCores.

Math notes:
  h = x @ W1.T + b1                      (computed in fp8 on the PE, fp32 accum)
  LIF (tau=2, v_th=1, hard reset to 0):
     v' = 0.5*v + 0.5*h
     s  = (v' >= 1);  v = v' * (v' < 1)
  Scan state is the pre-reset voltage w_t, kept with h pre-halved:
  hh = 0.5*h + 0.5*b1, and per step (one fused DVE op on VectorE):
     w  = select(w < 1, w, 0) * 0.5 + hh
  The spike complement m_t = (w_t < 1) is produced on the otherwise-idle
  GPSIMD engine as fp8 so the tensor engine can accumulate two steps per
  DoubleRow matmul: msum_psum += [I;I] @ [m_t; m_t+1], and
  sum_t s_t = T - msum.
  y = mean_t(s_t @ W2.T + b2) = (sum_t s_t) @ W2.T / T + b2
  out = log_softmax(y, axis=1)

Layout: the tensor engine contracts along the partition axis; the host packs
every input into its exact SBUF layout so each DMA is a plain contiguous
[128, N] copy with multi-KB per-partition lines:
  xT  [128, 16384] : free = chunk(4) * dj(8) * 512   (tb-major inside chunk)
  W1T [128, 8192]  : free = j(8) * dj(8) * 128       (output-tile major)
  W2T [128, 8192]  : free = ej(8) * 1024 (padded from 1000)
mm1 runs in 9 t-groups (7x4 steps + 2x2 steps) so the LIF scan starts early
and ends nearly with mm1; msum pairs trail two groups behind in the tensor
stream so the PE never waits on the scan.  PSUM->SBUF h copies (cast, *0.5,
+0.5*b1) stay on the scalar (ACT) engine which reads PSUM cheaply.
"""

import numpy as np
from contextlib import ExitStack

import concourse.bass as bass
import concourse.tile as tile
from concourse import bacc, mybir
from concourse.bass_utils import run_bass_kernel_spmd

N_CORES = 8


def _lif_op():
    """Fused LIF step as a custom DVE op:
         out = select(in0 < s0, in0, 0) * s1 + in1
       i.e. w_new = reset(w_old)*0.5 + hh  in a single VectorE instruction."""
    from concourse import dve_ops
    from concourse.dve_spec import Spec, Src0, Src1, Zero, C0, C1, select, lower
    from concourse.dve_uop import DveOpSpec

    for op in dve_ops.OPS:
        if op.name == "LIF_STEP_ANT":
            return op
    spec = Spec(
        body=select(Src0 < C0, Src0, Zero) * C1 + Src1,
        reference=lambda in0, in1, s0, s1, imm2: (
            np.where(in0.astype(np.float32) < s0, in0.astype(np.float32), 0.0) * s1
            + in1.astype(np.float32)).astype(np.float32),
    )
    row = dve_ops._CUSTOM_DVE_ROW_BASE + len(dve_ops.OPS)
    shas = {}
    for ver in ("v3", "v4"):
        try:
            shas[ver] = DveOpSpec(name="LIF_STEP_ANT", opcode=row,
                                  uops=lower(spec, ver=ver), rd1_en=True).sha(ver)
        except Exception:
            pass
    op = dve_ops.DveOp("LIF_STEP_ANT", spec, subdim=False, uops_sha=shas)
    dve_ops.OPS.append(op)
    dve_ops._SUB_OPCODE_FOR_NAME[op.name] = row
    dve_ops.CUSTOM_DVE_SPECS[op.name] = spec
    return op


T, B, D, C = 32, 512, 1024, 1000
BC = B // N_CORES          # 64 rows per core
TB = T * BC                # 2048 matmul rows per core
FP32 = mybir.dt.float32
BF16 = mybir.dt.bfloat16
FP8 = mybir.dt.float8e4
W1_PRESCALE = 256.0   # host multiplies W1/W2 by this (exact power of 2) so the
                      # small uniform(-1/32,1/32) values stay in fp8e4m3's
                      # normal range; compensated in the h-copy / y-copy scale
AF = mybir.ActivationFunctionType
OP = mybir.AluOpType

# mm1 t-groups: (t0, tcount), aligned to the 512-col x chunks.  Small groups
# at the start (scan spin-up) and end (scan tail); full-chunk FD512 groups in
# the middle where the DoubleRow LDWEIGHTS is fully hidden by the matmul.
GROUPS = [(0, 4), (4, 4), (8, 8), (16, 8), (24, 4), (28, 2), (30, 2)]


def build_program():
    nc = bacc.Bacc("TRN2", target_bir_lowering=False, debug=False, num_devices=N_CORES)

    xt_d = nc.dram_tensor("xT", [128, 16 * 1024], FP8, kind="ExternalInput").ap()
    w1t_d = nc.dram_tensor("W1T", [128, 8 * 1024], FP8, kind="ExternalInput").ap()
    b1_d = nc.dram_tensor("b1", [128, 8], FP32, kind="ExternalInput").ap()
    w2t_d = nc.dram_tensor("W2T", [128, 8 * 1024], FP8, kind="ExternalInput").ap()
    b2_d = nc.dram_tensor("b2", [C], FP32, kind="ExternalInput").ap()
    y_d = nc.dram_tensor("y", [BC, C], FP32, kind="ExternalOutput").ap()

    with tile.TileContext(nc) as tc, ExitStack() as ctx:
        persist = ctx.enter_context(tc.tile_pool(name="persist", bufs=1))
        small = ctx.enter_context(tc.tile_pool(name="small", bufs=1))
        ps_h = ctx.enter_context(tc.tile_pool(name="ps_h", bufs=5, space="PSUM"))
        ps_ms = ctx.enter_context(tc.tile_pool(name="ps_ms", bufs=1, space="PSUM"))
        ps_y = ctx.enter_context(tc.tile_pool(name="ps_y", bufs=2, space="PSUM"))

        # ---- input DMAs first so the HWDGE rings start streaming ASAP.
        # One ring sustains only ~150 GB/s, so the first-needed data (W1 j0/j1
        # + x chunk 0) is split across BOTH rings in need order.  The
        # scalar(ACT) ring gets few issues so the ACT queue is free for the
        # h-copies by the time mm1 group 0's PSUM is ready. ----
        xt = persist.tile([128, 16 * 1024], FP8)
        w1t = persist.tile([128, 8 * 1024], FP8)
        w2t = persist.tile([128, 8 * 1024], FP8)
        b1_sb = small.tile([128, 8], FP32)
        b2_sb = small.tile([1, C], FP32)

        nc.sync.dma_start(w1t[:, 0:1024], w1t_d[:, 0:1024])          # W1 j0
        nc.scalar.dma_start(xt[:, 2048:4096], xt_d[:, 2048:4096])    # x c0 dj4-7
        nc.sync.dma_start(xt[:, 0:2048], xt_d[:, 0:2048])            # x c0 dj0-3
        nc.scalar.dma_start(w1t[:, 4096:8192], w1t_d[:, 4096:8192])  # W1 j4-7
        nc.sync.dma_start(w1t[:, 1024:2048], w1t_d[:, 1024:2048])    # W1 j1
        nc.scalar.dma_start(b1_sb[:], b1_d[:])
        nc.scalar.dma_start(b2_sb[:], b2_d.rearrange("(a c) -> a c", a=1))
        nc.sync.dma_start(w1t[:, 2048:4096], w1t_d[:, 2048:4096])    # W1 j2-3
        nc.sync.dma_start(xt[:, 4096:8192], xt_d[:, 4096:8192])      # chunk 1
        nc.sync.dma_start(xt[:, 8192:12288], xt_d[:, 8192:12288])    # chunk 2
        nc.sync.dma_start(xt[:, 12288:16384], xt_d[:, 12288:16384])  # chunk 3
        nc.sync.dma_start(w2t[:], w2t_d[:])

        w1t4 = w1t[:].rearrange("p (j k e) -> p j k e", j=8, k=8)
        xt4 = xt[:].rearrange("p (c k t) -> p c k t", c=4, k=8)
        w2t3 = w2t[:].rearrange("p (e c) -> p e c", e=8)

        # ---- constants / biases (prologue, engines otherwise idle) ----
        io = small.tile([128, 128], mybir.dt.int32)
        nc.gpsimd.iota(io[:], pattern=[[1, 128]], base=0, channel_multiplier=-1)
        ones = small.tile([1, BC], BF16)
        nc.gpsimd.memset(ones[:], 1.0)
        ident = small.tile([128, 128], BF16)
        nc.vector.tensor_scalar(ident[:], io[:], 0, None, op0=OP.is_equal)

        b1h = small.tile([128, 8], FP32)
        nc.vector.tensor_scalar_mul(b1h[:], b1_sb[:], 0.5)
        b2_32 = small.tile([1, C], BF16)
        nc.scalar.activation(b2_32[:], b2_sb[:], AF.Copy,
                             scale=float(T) * W1_PRESCALE)
        warm = small.tile([1, 8], FP32)

        # ---- matmul1: h[e, tb] = W1 @ x.T, fused 0.5*h + 0.5*b1 into scan
        # layout via the ACT copy.  h_sb free index = t*512 + j*64 + b ----
        h_sb = persist.tile([128, T * 512], BF16)
        h3 = h_sb[:].rearrange("p (t x) -> p t x", x=512)

        def mm1_group(g, t0, tcnt):
            n = tcnt * 64
            c = (t0 * 64) // 512
            o = (t0 * 64) % 512
            for j in range(8):
                ps = ps_h.tile([128, 512], FP32, tag="ps_h", name=f"psh_{g}_{j}")
                for dp in range(4):   # pairs of contraction tiles (DoubleRow)
                    nc.tensor.matmul(
                        ps[:, 0:n],
                        w1t4[:, j, 2 * dp:2 * dp + 2, :],
                        xt4[:, c, 2 * dp:2 * dp + 2, o:o + n],
                        start=(dp == 0), stop=(dp == 3),
                        perf_mode=mybir.MatmulPerfMode.DoubleRow,
                    )
                nc.scalar.activation(
                    h3[:, t0:t0 + tcnt, j * 64:(j + 1) * 64],
                    ps[:, 0:n].rearrange("p (t b) -> p t b", t=tcnt),
                    AF.Identity, scale=0.5 / W1_PRESCALE, bias=b1h[:, j:j + 1],
                )

        # ---- spike-sum accumulation: msum += I @ m_t ----
        m_all = persist.tile([128, T * 512], BF16)
        msum = ps_ms.tile([128, 512], FP32)

        def ms_group(t0, tcnt):
            for t in range(t0, t0 + tcnt):
                nc.tensor.matmul(msum[:], ident[:],
                                 m_all[:, t * 512:(t + 1) * 512],
                                 start=(t == 0), stop=(t == T - 1))

        # ---- LIF scan: VectorE custom op + bf16 mask on the same queue ----
        lif = _lif_op()
        wst = small.tile([128, 512], BF16)
        nc.vector.memset(wst[:], 0.0)

        def scan_steps(t0, tcnt):
            for t in range(t0, t0 + tcnt):
                h_t = h_sb[:, t * 512:(t + 1) * 512]
                nc.vector._custom_dve(lif, out=wst[:], in0=wst[:], in1=h_t,
                                      s0=1.0, s1=0.5)
                nc.vector.tensor_scalar(m_all[:, t * 512:(t + 1) * 512],
                                        wst[:], 1.0, None, op0=OP.is_lt)

        def warm_mm(n=512):
            ps = ps_h.tile([128, 512], FP32, tag="ps_h", name="warm")
            src = m_all[:, 0:n] if n == 512 else ident[:]
            nc.tensor.matmul(ps[:, 0:n], ident[:], src, start=True, stop=True)

        # PE p-state pre-warm: a bridge of dummy matmuls so group 0 starts at
        # full clock instead of paying the 0.65/1.2 GHz ramp while data lands.
        for _ in range(14):
            warm_mm(128)

        # Emission is program order: scan steps trail mm1 by one group (their
        # h is complete), msum matmuls trail by two (their masks are complete),
        # so the tensor stream stays dense and never waits on the scan until
        # the very tail.
        ng = len(GROUPS)
        for g in range(ng):
            mm1_group(g, *GROUPS[g])
            if g == 3:
                # warm the Exp/Ln ACT tables mid-kernel (ACT has slack here)
                # so the epilogue doesn't pay the ~1.3us table-load switches.
                # Ln first so the pass leaves the exp set resident for Exp.
                nc.scalar.activation(warm[:, 4:8], b1_sb[0:1, 4:8], AF.Ln)
                nc.scalar.activation(warm[:, 0:4], b1_sb[0:1, 0:4], AF.Exp)
            if g >= 1:
                scan_steps(*GROUPS[g - 1])
            if 2 <= g <= ng - 2:
                ms_group(*GROUPS[g - 2])

        # mm2 bias rank-1 matmuls emitted here: no scan dependency, and they
        # keep the PE busy (p-state) while the scan tail drains.  The dummy
        # matmuls interleaved with the tail ms reread old mask slices purely
        # to keep the PE's p-state ramped for mm2.
        psy = [ps_y.tile([BC, 512], FP32, tag="ps_y", name=f"psy{h}")
               for h in range(2)]
        for half in range(2):
            n = 512 if half == 0 else C - 512
            c0 = half * 512
            nc.tensor.matmul(psy[half][:, 0:n], ones[:], b2_32[:, c0:c0 + n],
                             start=True, stop=False)

        scan_steps(*GROUPS[ng - 1])
        for t in range(GROUPS[ng - 3][0], 32):
            nc.tensor.matmul(msum[:], ident[:], m_all[:, t * 512:(t + 1) * 512],
                             start=False, stop=(t == T - 1))
            if t < T - 1:
                warm_mm()
                warm_mm()
                warm_mm()

        # sum_t s_t = T - msum; spike counts are small integers, ~exact in fp8
        ssum = small.tile([128, 512], FP8)
        nc.scalar.activation(ssum[:], msum[:], AF.Copy, scale=-1.0, bias=float(T))
        ssum3 = ssum[:].rearrange("p (j b) -> p j b", j=8)

        # ---- matmul2: y = ssum @ W2.T / T + b2 (DoubleRow fp8), kept in
        # PSUM; the epilogue reads PSUM directly (no y_sb staging copy) ----
        for half in range(2):
            n = 512 if half == 0 else C - 512
            c0 = half * 512
            for pj in range(4):
                nc.tensor.matmul(
                    psy[half][:, 0:n],
                    ssum3[:, 2 * pj:2 * pj + 2, :],
                    w2t3[:, 2 * pj:2 * pj + 2, c0:c0 + n],
                    start=False, stop=(pj == 3),
                    perf_mode=mybir.MatmulPerfMode.DoubleRow,
                )

        # ---- log_softmax over C (y is small enough that no max-shift is
        # needed: |y| <= D/T + |b2| ~ 32, exp stays in fp32 range).  The Exp
        # accumulator gives the row sums without a separate DVE reduce; the
        # final subtract fuses the 1/(T*PRESCALE) scale on the PSUM read. ----
        ysc = 1.0 / (T * W1_PRESCALE)
        ez = small.tile([BC, 1024], FP32)
        se = small.tile([BC, 2], FP32)
        nc.scalar.activation(ez[:, 0:512], psy[0][:, 0:512], AF.Exp,
                             scale=ysc, accum_out=se[:, 0:1])
        nc.scalar.activation(ez[:, 512:C], psy[1][:, 0:C - 512], AF.Exp,
                             scale=ysc, accum_out=se[:, 1:2])
        ssum_e = small.tile([BC, 1], FP32)
        nc.vector.tensor_scalar(ssum_e[:], se[:, 0:1], se[:, 1:2], None,
                                op0=OP.add)
        lse = small.tile([BC, 1], FP32)
        nc.scalar.activation(lse[:], ssum_e[:], AF.Ln)
        out_sb = small.tile([BC, C], FP32)
        nc.vector.tensor_scalar(out_sb[:, 0:512], psy[0][:, 0:512], ysc,
                                lse[:], op0=OP.mult, op1=OP.subtract)
        nc.sync.dma_start(y_d[:, 0:512], out_sb[:, 0:512])
        nc.vector.tensor_scalar(out_sb[:, 512:C], psy[1][:, 0:C - 512], ysc,
                                lse[:], op0=OP.mult, op1=OP.subtract)
        nc.sync.dma_start(y_d[:, 512:C], out_sb[:, 512:C])

    nc.compile()
    return nc


_CACHE = {}


def kernel(x, W1, b1, W2, b2):
    if "nc" not in _CACHE:
        _CACHE["nc"] = build_program()
    nc = _CACHE["nc"]

    f8 = mybir.dt.np(FP8)
    x = np.asarray(x, dtype=np.float32)
    # W1T packed [p, j(8), dj(8), 128]: W1T[dj*128+p, j*128+e'] -> [p][j][dj][e']
    w1f8 = (np.asarray(W1, dtype=np.float32).T * W1_PRESCALE).astype(f8)
    w1p = np.ascontiguousarray(
        w1f8.reshape(8, 128, 8, 128).transpose(1, 2, 0, 3).reshape(128, 8192))
    # W2T packed [p, ej(8), 1024 (padded from 1000)]
    w2f8 = (np.asarray(W2, dtype=np.float32).T * W1_PRESCALE).astype(f8)
    w2p = np.zeros((128, 8, 1024), dtype=f8)
    w2p[:, :, 0:C] = w2f8.reshape(8, 128, C).transpose(1, 0, 2)
    w2p = np.ascontiguousarray(w2p.reshape(128, 8192))
    # b1 pre-transposed to the [p, j] SBUF layout (contiguous DMA lines)
    b1 = np.ascontiguousarray(
        np.asarray(b1, dtype=np.float32).reshape(8, 128).T)
    b2 = np.ascontiguousarray(b2, dtype=np.float32)

    in_maps = []
    for i in range(N_CORES):
        # xT packed [p, chunk(4), dj(8), 512]: xT[dj*128+p, c*512+t']
        xs8 = x[:, i * BC:(i + 1) * BC, :].reshape(TB, D).astype(f8)
        xs = np.ascontiguousarray(
            xs8.T.reshape(8, 128, 4, 512).transpose(1, 2, 0, 3).reshape(128, 16384))
        in_maps.append({"xT": xs, "W1T": w1p, "b1": b1, "W2T": w2p, "b2": b2})

    res = run_bass_kernel_spmd(nc, in_maps, core_ids=list(range(N_CORES)),
                               **_CACHE.get("run_kwargs", {}))
    _CACHE["last_results"] = res
    out = np.concatenate([res.results[i]["y"] for i in range(N_CORES)], axis=0)
    return out
